# revision 80
# baseline (speedup 1.0000x reference)
"""DBHead (non-local attention + binarize/threshold conv branches) on 8 trn2 cores.

Sharding: 8 shards = 4 batch x 2 row-halves. Core (b, s) computes output rows
[128s, 128s+128) of the [3, 256, 256] map for batch b. All per-core variation
(which rows, halo padding, query-row masking) is pushed into host-prepared
input data so ONE SPMD program serves all 8 cores.

Final schedule (TimelineSim 179.9us/core vs 235.9us baseline, HW-verified
rel err 1.8e-3):
- everything DMA'd straight into f16 tiles (no staging/conversion pass);
  all weight/activation matmuls run f16 at full PE rate. Only exp outputs
  (range ~e^35) and V stay f32r.
- threshold-branch inputs load first; junk warm-up matmuls (high priority)
  ramp the PE clock while those DMAs fly.
- Act-engine ops are the scarce resource after the PE: every PSUM
  eviction is batched 2 banks per Act op (e1/e2 pairs, V quads, score
  pairs, 8-partition output strips) so the Act period stays under the
  PE period for each pipeline.
- attention runs as one flat software-pipelined stream over all (q-block,
  key-group) stages; S@V lags score/exp by 2 stages and a 64-wide pacing
  matmul per stage keeps the PE period just above the Act exp period so
  the exp pipeline never drifts into its PSUM WAR. The softmax
  denominator chain (DVE/Pool accumulate, gpsimd partition_all_reduce,
  reciprocal * query mask) runs entirely off the PE path; the final
  q-block's chain lives in long-lived tiles behind a fast unnormalized
  PSUM copy so no phase-3 pool barrier waits on it.
- ConvT taps packed two-per-matmul (M=64 -> 128) in dw1 and dw2; the
  binarize-threshold diff + sigmoid work is deferred off the Act critical
  path, the threshold map never leaves SBUF, and each output channel
  ships as ONE DMA descriptor (DRAM layout = SBUF accumulator layout).
"""
import sys, os
sys.path.insert(0, "/opt/trn_rl_repo")
import numpy as np
from contextlib import ExitStack

import concourse.bass as bass
import concourse.tile as tile
from concourse import mybir, bacc
from concourse.bass_utils import run_bass_kernel_spmd
from concourse import bass_isa

F32 = mybir.dt.float32
F32R = mybir.dt.float32r
F16 = mybir.dt.float16
AFT = mybir.ActivationFunctionType
ALU = mybir.AluOpType

EPS = 1e-5
NQ = 2176  # 34 rows x 64 cols of query positions (33 real + 1 zero halo row)
QBLOCKS = [(0, 512), (512, 512), (1024, 512), (1536, 512), (2048, 128)]
NKC = 32  # key chunks of 128 over 4096 positions

# wpack column offsets (f16, rows = contraction dim on partitions).
# Order = load priority: threshold branch first, then binarize, then attn.
OFF_THW = 0        # 18 x 64 (tap*2+chunk)
OFF_DW1TH = 1152   # 2 tap-pairs x 128 (rows 0:64)
OFF_DW2TH = 1408   # 8 cols block-diag (rows 0:128)
OFF_BZW = 1416
OFF_DW1BZ = 2568
OFF_DW2BZ = 2824
OFF_WQ = 2832      # 2 chunks x 64
OFF_WK = 2960
OFF_WA = 3088      # 2 chunks x 256
OFF_ONESR = 3600   # 128 cols of ones (row0 = K=1 bcast lhsT)
WCOLS = 3728

# bpack columns
BP_BQ, BP_BK = 0, 1
BP_BZ_S1, BP_BZ_B1, BP_BZ_S2, BP_BZ_B2 = 2, 3, 4, 5
BP_TH_S1, BP_TH_B1, BP_TH_S2, BP_TH_B2 = 6, 7, 8, 9
BP_BZ_DB2, BP_TH_DB2 = 10, 11
BCOLS = 16

_CACHE = {}
LAST_RESULTS = None


def _branch_ir(nc, tc, wr, bpk, hc, pcv, pct, ppt, pads, offw3, s1, b1,
               offdw1, s2, b2, offdw2, db2, strip_fn, pre_blk_fn=None,
               blk_order=(0, 1, 2, 3), late_wait_ms=None):
    """Conv3x3+BN+ReLU, ConvT(2x2) x2 pipeline over 4 blocks of 8 rows.
    strip_fn(blk, pt, last) consumes each finished [8, 2, 512] PSUM strip
    pair (partitions 0:4 = dw1-tap 2p*4.., free = (p, col)).
    pre_blk_fn() can inject deferred work between blocks.
    """
    h1cs, h2cs = {}, {}

    def stage_a(blk, act_h1=False):  # conv3x3 into PSUM + BN/ReLU eviction
        cv = pcv.tile([64, 512], F32, tag="cv", name="cv")
        for t in range(9):
            ky, kx = t // 3, t % 3
            for c in range(2):
                o = offw3 + (t * 2 + c) * 64
                nc.tensor.matmul(
                    cv[:], lhsT=wr[:, o:o + 64],
                    rhs=pads[c][:, blk * 8 + ky:blk * 8 + ky + 8, kx:kx + 64],
                    start=(t == 0 and c == 0), stop=(t == 8 and c == 1))
        h1c = hc.tile([64, 512], F16, tag="h1c", name="h1c")
        if act_h1:
            # fused BN+ReLU on Act: used when the DVE queue is backlogged
            nc.scalar.activation(h1c[:], cv[:], AFT.Relu,
                                 bias=bpk[0:64, b1:b1 + 1],
                                 scale=bpk[0:64, s1:s1 + 1])
        else:
            h1t = hc.tile([64, 512], F32, tag="h1t", name="h1t")
            nc.vector.tensor_scalar(h1t[:], cv[:], bpk[0:64, s1:s1 + 1],
                                    bpk[0:64, b1:b1 + 1], ALU.mult, ALU.add)
            nc.vector.tensor_scalar_max(h1c[:], h1t[:], 0.0)
        h1cs[blk] = h1c

    def stage_b(blk):  # dw1 tap-pair matmuls + one paired BN/ReLU eviction
        ct = pct.tile([128, 2, 512], F32, tag="ct", name="ct")
        for p in range(2):
            o = offdw1 + p * 128
            nc.tensor.matmul(ct[:, p, :], lhsT=wr[0:64, o:o + 128],
                             rhs=h1cs[blk][:], start=True, stop=True)
        h2c = hc.tile([128, 2, 512], F16, tag="h2c", name="h2c")
        nc.scalar.activation(h2c[:], ct[:], AFT.Relu,
                             bias=bpk[:, b2:b2 + 1],
                             scale=bpk[:, s2:s2 + 1])
        h2cs[blk] = h2c

    def stage_c(blk, last=False):  # dw2 matmuls + output strip
        if pre_blk_fn is not None:
            pre_blk_fn()
        pt = ppt.tile([8, 2, 512], F32, tag="pt", name="pt")
        for p in range(2):
            nc.tensor.matmul(pt[:, p, :], lhsT=wr[:, offdw2:offdw2 + 8],
                             rhs=h2cs[blk][:, p, :], start=True, stop=True)
        strip_fn(blk, pt, last)

    # software-pipelined emission: the PE always has a later block's conv
    # between a dw1 and the dw2 that consumes its Act-evicted h2c.
    o = blk_order
    act_h1 = blk_order[0] != 0  # reversed = binarize branch after attention
    stage_a(o[0], act_h1); stage_a(o[1], act_h1); stage_b(o[0])
    stage_a(o[2]); stage_c(o[0]); stage_b(o[1])
    if late_wait_ms is not None:
        # keep the scheduler from hoisting this conv (whose input is the
        # final attention rows) ahead of the rest of the branch
        with tc.tile_wait_until(late_wait_ms):
            stage_a(o[3])
    else:
        stage_a(o[3])
    stage_c(o[1]); stage_b(o[2]); stage_b(o[3])
    stage_c(o[2]); stage_c(o[3], last=True)


def _build():
    nc = bacc.Bacc("TRN2", target_bir_lowering=False, debug=False, num_devices=8)
    xin_d = nc.dram_tensor("xin", [256, 64, 64], F16, kind="ExternalInput").ap()
    xq_d = nc.dram_tensor("xq", [256, 34, 64], F16, kind="ExternalInput").ap()
    xpad_d = nc.dram_tensor("xpad", [256, 34, 66], F16, kind="ExternalInput").ap()
    qm_d = nc.dram_tensor("qmask", [1, NQ], F32, kind="ExternalInput").ap()
    wp_d = nc.dram_tensor("wpack", [128, WCOLS], F16, kind="ExternalInput").ap()
    bp_d = nc.dram_tensor("bpack", [128, BCOLS], F32, kind="ExternalInput").ap()
    ba_d = nc.dram_tensor("ba", [256], F16, kind="ExternalInput").ap()
    out_d = nc.dram_tensor("out", [3, 8, 4, 2, 512], F32, kind="ExternalOutput").ap()

    with tile.TileContext(nc) as tc, ExitStack() as ctx:
        cp = ctx.enter_context(tc.tile_pool(name="const", bufs=1))
        pp = ctx.enter_context(tc.tile_pool(name="pads", bufs=1))

        wr = cp.tile([128, WCOLS], F16)
        bpk = cp.tile([128, BCOLS], F32)
        bar = cp.tile([1, 256], F16)
        qm128 = cp.tile([128, NQ], F32)
        zc = cp.tile([128, 34], F32)
        # threshold map kept on-chip: partitions = 4*(tap-in-pair i) + convT2
        # tap, free = (blk, dw1-pair p, col); output tap t = 2p + i
        Ts = cp.tile([8, 4, 2, 512], F32)

        xpr = [pp.tile([128, 34, 66], F16, tag=f"xp{c}", name=f"xpr{c}")
               for c in range(2)]
        xnp = [pp.tile([128, 34, 66], F16, tag=f"xn{c}", name=f"xnp{c}")
               for c in range(2)]
        # the FINAL attention q-block's normalization chain lives here (not
        # in a phase-2-scoped pool) so the phase-3 pool-close barriers never
        # wait on it; its PSUM is freed by one fast copy into xnr2.
        xnr2 = pp.tile([128, 2, 512], F32, tag="xnr2", name="xnr2")
        A2f = pp.tile([128, 2, 512], F32, tag="A2f", name="A2f")
        Fs2 = pp.tile([128, 512], F32, tag="Fs2", name="Fs2")
        Rs2 = pp.tile([128, 512], F32, tag="Rs2", name="Rs2")
        Ci2 = pp.tile([128, 512], F32, tag="Ci2", name="Ci2")
        Cm2 = pp.tile([128, 512], F32, tag="Cm2", name="Cm2")

        # ---- loads, in priority order (threshold-branch inputs first).
        # All issued from the idle SP engine. ----
        qm = cp.tile([1, NQ], F32)
        nc.sync.dma_start(wr[:, 0:OFF_BZW], wp_d[:, 0:OFF_BZW])
        for c in range(2):
            sl = slice(c * 128, (c + 1) * 128)
            nc.sync.dma_start(xpr[c][:], xpad_d[sl])
        nc.sync.dma_start(bpk[:], bp_d[:])
        nc.sync.dma_start(wr[:, OFF_BZW:], wp_d[:, OFF_BZW:])
        nc.sync.dma_start(qm[:], qm_d[:])
        nc.sync.dma_start(bar[:], bass.AP(tensor=ba_d.tensor, offset=ba_d.offset,
                                          ap=[[0, 1]] + [list(a) for a in ba_d.ap]))

        nc.vector.memset(zc[:], 0.0)
        # ba broadcast to all partitions x 4 value-chunks, for the DVE-side
        # V evictions that fold the bias in (Act bias is per-partition only)
        baf4 = cp.tile([1, 4, 256], F32)
        bav4 = cp.tile([128, 4, 256], F32)
        for u in range(4):
            nc.vector.tensor_copy(baf4[:, u, :], bar[:])
        nc.gpsimd.partition_broadcast(bav4[:], baf4[:])

        # ---- PE clock warm-up: junk matmuls while the first DMAs fly ----
        with tc.high_priority(), \
             tc.tile_pool(name="wrm", bufs=1) as wp_, \
             tc.tile_pool(name="wps", bufs=2, space="PSUM") as wps:
            jz = wp_.tile([128, 512], F16)
            nc.vector.memset(jz[:], 0.0)
            jo = wp_.tile([1, 16], F32)
            for i in range(9):
                wt = wps.tile([128, 512], F32, tag="w", name="wt")
                nc.tensor.matmul(wt[:], lhsT=jz[:, 0:128], rhs=jz[:],
                                 start=True, stop=True)
                # keep the BIR verifier happy: every PSUM write needs a reader
                nc.vector.tensor_copy(jo[:, i:i + 1], wt[0:1, 0:1])

        with tc.tile_pool(name="att", bufs=1) as ap_:
            e1r = ap_.tile([64, NQ], F16)
            e2r = ap_.tile([64, 4096], F16)
            V = ap_.tile([128, NKC, 256], F32R)

            with tc.tile_pool(name="xr", bufs=1) as xp:
                xr = [xp.tile([128, 64, 64], F16, tag=f"xr{c}", name=f"xr{c}")
                      for c in range(2)]
                xqr = [xp.tile([128, 34, 64], F16, tag=f"xq{c}", name=f"xqr{c}")
                       for c in range(2)]
                for c in range(2):
                    sl = slice(c * 128, (c + 1) * 128)
                    nc.sync.dma_start(xqr[c][:], xq_d[sl])
                for c in range(2):
                    sl = slice(c * 128, (c + 1) * 128)
                    nc.sync.dma_start(xr[c][:], xin_d[sl])

                # threshold branch: independent of attention; fills the PE
                # while attention inputs stream in. Strips go to DRAM (f32)
                # and to the on-chip f16 Tt for the binarize diff later.
                with tc.tile_pool(name="hct", bufs=2) as hct, \
                     tc.tile_pool(name="pcv0", bufs=2, space="PSUM") as pcv0, \
                     tc.tile_pool(name="pct0", bufs=2, space="PSUM") as pct0, \
                     tc.tile_pool(name="ppt0", bufs=1, space="PSUM") as ppt0:
                    def th_strip(blk, pt, last):
                        nc.scalar.activation(
                            Ts[:, blk, :, :], pt[:], AFT.Sigmoid,
                            bias=bpk[0:8, BP_TH_DB2:BP_TH_DB2 + 1])
                    _branch_ir(nc, tc, wr, bpk, hct, pcv0, pct0, ppt0, xpr,
                               OFF_THW, BP_TH_S1, BP_TH_B1, OFF_DW1TH,
                               BP_TH_S2, BP_TH_B2, OFF_DW2TH, BP_TH_DB2,
                               th_strip)
                    # DRAM layout matches the SBUF tile: one descriptor
                    nc.scalar.dma_start(out_d[1], Ts[:])
                # zero borders of xn pads (cols 0/65); rows are written later
                for c in range(2):
                    for col in (0, 65):
                        nc.vector.tensor_copy(
                            xnp[c][:, :, col:col + 1],
                            zc[:].rearrange("p (r o) -> p r o", o=1))

                xr_f = [t[:].rearrange("p r c2 -> p (r c2)") for t in xr]
                xq_f = [t[:].rearrange("p r c2 -> p (r c2)") for t in xqr]

                # ---- phase 1: e1 (queries), e2 (keys), V (values).
                # Two 512-chunks per PSUM tile so each Act eviction covers
                # 2 banks and the Act period stays under the PE period. ----
                with tc.tile_pool(name="pe", bufs=2, space="PSUM") as pe, \
                     tc.tile_pool(name="pv", bufs=2, space="PSUM") as pv:
                    def proj(dst, src_f, off, bias_col, k0, w2):
                        # one [64, 2, 512] psum tile covering cols k0:k0+w2
                        p = pe.tile([64, 2, 512], F32, tag="pe", name="pe_t")
                        for h in range(2):
                            hw = min(512, w2 - h * 512)
                            if hw <= 0:
                                break
                            for c in range(2):
                                o = off + c * 64
                                nc.tensor.matmul(
                                    p[:, h, :hw], lhsT=wr[:, o:o + 64],
                                    rhs=src_f[c][:, k0 + h * 512:k0 + h * 512 + hw],
                                    start=(c == 0), stop=(c == 1))
                        nc.scalar.activation(
                            dst[:, k0:k0 + w2].rearrange("p (h c2) -> p h c2",
                                                         c2=512)
                            if w2 == 1024 else dst[:, k0:k0 + w2],
                            p[:, :, :] if w2 == 1024 else p[:, 0, :w2],
                            AFT.Prelu, bias=bpk[0:64, bias_col:bias_col + 1],
                            alpha=0.25)
                    for k0 in range(0, 2048, 1024):
                        proj(e1r, xq_f, OFF_WQ, BP_BQ, k0, 1024)
                    proj(e1r, xq_f, OFF_WQ, BP_BQ, 2048, 128)
                    for q in range(NKC // 4):  # V in quads: 4 chunks/eviction
                        act_q = q % 2 == 0
                        p = pv.tile([128, 4, 256], F32, tag="pv", name="pv_t")
                        for u in range(4):
                            j = 4 * q + u
                            for c in range(2):
                                o = OFF_WA + c * 256
                                nc.tensor.matmul(p[:, u, :],
                                                 lhsT=xr_f[c][:, j * 128:(j + 1) * 128],
                                                 rhs=wr[:, o:o + 256],
                                                 start=(c == 0),
                                                 stop=(c == 1 and not act_q))
                            if act_q:
                                # Act's Prelu bias is per-partition only, so
                                # these quads take the bias via a ones-matmul
                                nc.tensor.matmul(
                                    p[:, u, :],
                                    lhsT=wr[0:1, OFF_ONESR:OFF_ONESR + 128],
                                    rhs=bar[:], start=False, stop=True)
                        if act_q:
                            nc.scalar.activation(V[:, 4 * q:4 * q + 4, :], p[:],
                                                 AFT.Prelu, alpha=0.25)
                        else:
                            # DVE eviction folds the bias in: z = p + ba
                            # (broadcast), then PReLU(z) == max(0.25z, z).
                            # One PSUM input per instruction (HW rule).
                            sl = V[:, 4 * q:4 * q + 4, :]
                            nc.vector.scalar_tensor_tensor(
                                sl, p[:], 1.0, bav4[:], ALU.mult, ALU.add)
                            nc.vector.scalar_tensor_tensor(
                                sl, sl, 0.25, sl, ALU.mult, ALU.max)
                    # e2 last: its first chunk-pair unblocks the score
                    # matmuls while the remaining evictions drain
                    for k0 in range(0, 4096, 1024):
                        proj(e2r, xr_f, OFF_WK, BP_BK, k0, 1024)

            # ---- phase 2: attention, one flat pipelined stream over all
            # (q-block, key-group) stages; consume side lags produce side
            # by 2 stages so the PE never waits on exp. Big blocks use
            # 2-key-chunk stages; the small tail block packs 8 key chunks
            # per stage so its exp overhead stays amortized. ----
            nc.gpsimd.partition_broadcast(qm128[:], qm[:])
            stages = []
            for q0, w in QBLOCKS:
                kper = 2 if w > 128 else 8
                for g in range(NKC // kper):
                    stages.append((q0, w, list(range(kper * g, kper * g + kper))))
            LAG = 2
            with tc.tile_pool(name="psc", bufs=2, space="PSUM") as psc, \
                 tc.tile_pool(name="pxn", bufs=2, space="PSUM") as pxn, \
                 tc.tile_pool(name="eb", bufs=10) as eb, \
                 tc.tile_pool(name="rc", bufs=2) as rc:
                Es = {}
                cur = {}
                for i in range(len(stages) + LAG):
                    if i < len(stages):
                        q0, w, ks = stages[i]
                        sc = psc.tile([128, 2, 512], F32, tag="sc", name="sc")
                        if w == 512:
                            # pacing matmul (+27ns, overwritten by the real
                            # score below): keeps the PE stage period just
                            # above the Act exp period so the exp pipeline
                            # never drifts into the PSUM WAR stall
                            nc.tensor.matmul(sc[:, 0, 0:64],
                                             lhsT=e2r[:, 0:128],
                                             rhs=e1r[:, q0:q0 + 64],
                                             start=True, stop=True)
                        for n, j in enumerate(ks):
                            u, m = n % 2, n // 2
                            nc.tensor.matmul(sc[:, u, m * w:(m + 1) * w],
                                             lhsT=e2r[:, j * 128:(j + 1) * 128],
                                             rhs=e1r[:, q0:q0 + w],
                                             start=True, stop=True)
                        E = eb.tile([128, 2, 512], F32R, tag="E", name="E")
                        wv = w * len(ks) // 2
                        nc.scalar.activation(E[:, :, :wv], sc[:, :, :wv], AFT.Exp)
                        Es[i] = E
                    if i >= LAG:
                        k = i - LAG
                        q0, w, ks = stages[k]
                        wv = w * len(ks) // 2
                        first = ks[0] == 0
                        if first:
                            cur["xn"] = pxn.tile([128, 2, 512], F32, tag="xn",
                                                 name="xnps")
                            cur["A"] = (A2f if q0 == QBLOCKS[-1][0] else
                                        rc.tile([128, 2, 512], F32, tag="A",
                                                name="A"))
                        xn_ps, A = cur["xn"], cur["A"]
                        E = Es.pop(k)
                        final = (ks[-1] == NKC - 1 and q0 == QBLOCKS[-1][0])
                        for n, j in enumerate(ks):
                            u, m = n % 2, n // 2
                            for t in range(2):
                                nc.tensor.matmul(
                                    xn_ps[:, t, :w],
                                    lhsT=V[:, j, t * 128:(t + 1) * 128],
                                    rhs=E[:, u, m * w:m * w + w],
                                    start=(j == 0), stop=(j == NKC - 1))
                        if final:
                            # evict unnormalized BEFORE the remaining adds so
                            # the phase-3 PSUM pool barrier releases at once
                            with tc.high_priority():
                                nc.vector.tensor_copy(xnr2[:, :, :w],
                                                      xn_ps[:, :, :w])
                        # DVE adds are ~3.5x faster than Pool's: ~1 in 5
                        # accumulates goes to Pool, evenly spread
                        gi = ks[0] // len(ks)
                        eng = nc.gpsimd if gi % 5 == 2 else nc.vector
                        if first:
                            eng.tensor_copy(A[:, :, :wv], E[:, :, :wv])
                        else:
                            eng.tensor_add(A[:, :, :wv], A[:, :, :wv],
                                           E[:, :, :wv])
                        if ks[-1] == NKC - 1 and not final:
                            rows, r0 = w // 64, q0 // 64
                            Fs = rc.tile([128, 512], F32, tag="Fs", name="Fs")
                            nc.vector.tensor_add(Fs[:, :wv], A[:, 0, :wv],
                                                 A[:, 1, :wv])
                            # fold sub-slots (small tail block only)
                            while wv > w:
                                wv //= 2
                                nc.vector.tensor_add(Fs[:, :wv], Fs[:, :wv],
                                                     Fs[:, wv:2 * wv])
                            Rs = rc.tile([128, 512], F32, tag="Rs", name="Rs")
                            nc.gpsimd.partition_all_reduce(
                                Rs[:, :w], Fs[:, :w], 128, bass_isa.ReduceOp.add)
                            Ci = rc.tile([128, 512], F32, tag="Ci", name="Ci")
                            nc.vector.reciprocal(Ci[:, :w], Rs[:, :w])
                            Cm = rc.tile([128, 512], F32, tag="Cm", name="Cm")
                            nc.vector.tensor_mul(Cm[:, :w], Ci[:, :w],
                                                 qm128[:, q0:q0 + w])
                            for t in range(2):
                                # gpsimd cannot touch PSUM: both on DVE
                                nc.vector.tensor_mul(
                                    xnp[t][:, r0:r0 + rows, 1:65],
                                    xn_ps[:, t, :w].rearrange(
                                        "p (r c2) -> p r c2", c2=64),
                                    Cm[:, :w].rearrange(
                                        "p (r c2) -> p r c2", c2=64))

            # final q-block's normalization: emitted after the phase-2 pools
            # close (only long-lived tiles involved) so no pool-close barrier
            # waits on it; the binarize branch reads these rows ~10us later.
            q0f, wf = QBLOCKS[-1]
            rowsf, r0f = wf // 64, q0f // 64
            wvf = wf * (8 if wf == 128 else 2) // 2
            nc.vector.tensor_add(Fs2[:, :wvf], A2f[:, 0, :wvf],
                                 A2f[:, 1, :wvf])
            while wvf > wf:  # fold sub-slots (small tail block only)
                wvf //= 2
                nc.vector.tensor_add(Fs2[:, :wvf], Fs2[:, :wvf],
                                     Fs2[:, wvf:2 * wvf])
            nc.gpsimd.partition_all_reduce(Rs2[:, :wf], Fs2[:, :wf], 128,
                                           bass_isa.ReduceOp.add)
            nc.vector.reciprocal(Ci2[:, :wf], Rs2[:, :wf])
            nc.vector.tensor_mul(Cm2[:, :wf], Ci2[:, :wf],
                                 qm128[:, q0f:q0f + wf])
            for t in range(2):
                nc.vector.tensor_mul(
                    xnp[t][:, r0f:r0f + rowsf, 1:65],
                    xnr2[:, t, :wf].rearrange("p (r c2) -> p r c2", c2=64),
                    Cm2[:, :wf].rearrange("p (r c2) -> p r c2", c2=64))

        # ---- phase 3: binarize branch (att pool closed; SBUF freed).
        # P strips stream out as they finish; the threshold diff + final
        # sigmoid (B channel) is deferred off the Act critical path. ----
        with tc.tile_pool(name="hc", bufs=4) as hc, \
             tc.tile_pool(name="pb", bufs=1) as pb, \
             tc.tile_pool(name="pcv", bufs=2, space="PSUM") as pcv, \
             tc.tile_pool(name="pct", bufs=1, space="PSUM") as pct, \
             tc.tile_pool(name="ppt", bufs=2, space="PSUM") as ppt:
            Pa = pb.tile([8, 4, 2, 512], F32, tag="Pa", name="Pa")
            Ba = pb.tile([8, 4, 2, 512], F32, tag="Ba", name="Ba")
            pending = []

            def flush_one():
                blk, dc = pending.pop(0)
                nc.scalar.activation(Ba[:, blk, :, :], dc[:], AFT.Sigmoid,
                                     scale=50.0)

            def bz_strip(blk, pt, last):
                if not last:
                    nc.scalar.activation(Pa[:, blk, :, :], pt[:], AFT.Sigmoid,
                                         bias=bpk[0:8, BP_BZ_DB2:BP_BZ_DB2 + 1])
                    dc = hc.tile([8, 2, 512], F32, tag="dc", name="dc")
                    nc.vector.tensor_sub(dc[:], Pa[:, blk, :, :],
                                         Ts[:, blk, :, :])
                    pending.append((blk, dc))
                    return
                # tail block: drain deferred sigmoids first (the row DMAs
                # below read every block), then per-pair pieces so the
                # final Act chain is short
                while pending:
                    flush_one()
                for p in range(2):
                    nc.scalar.activation(Pa[:, blk, p, :], pt[:, p, :],
                                         AFT.Sigmoid,
                                         bias=bpk[0:8, BP_BZ_DB2:BP_BZ_DB2 + 1])
                    dcp = hc.tile([8, 512], F32, tag="dcp", name="dcp")
                    nc.vector.tensor_sub(dcp[:], Pa[:, blk, p, :],
                                         Ts[:, blk, p, :])
                    nc.scalar.activation(Ba[:, blk, p, :], dcp[:], AFT.Sigmoid,
                                         scale=50.0)
                    if p == 1:
                        # all blocks complete: single-descriptor channels
                        nc.sync.dma_start(out_d[0], Pa[:])
                        nc.scalar.dma_start(out_d[2], Ba[:])

            def pre_blk():
                while len(pending) >= 1:
                    flush_one()

            _branch_ir(nc, tc, wr, bpk, hc, pcv, pct, ppt, xnp, OFF_BZW,
                       BP_BZ_S1, BP_BZ_B1, OFF_DW1BZ, BP_BZ_S2, BP_BZ_B2,
                       OFF_DW2BZ, BP_BZ_DB2, bz_strip, pre_blk_fn=pre_blk,
                       blk_order=(0, 1, 2, 3))
            while pending:
                flush_one()

    nc.compile()
    return nc


def _prep(inputs):
    """Host-side parameter prep shared by all cores (numpy, tiny)."""
    g = {k: np.asarray(v, np.float32) for k, v in inputs.items()}
    wpack = np.zeros((128, WCOLS), np.float32)
    wqT = g["wm1"].reshape(64, 256).T
    wpack[:, OFF_WQ:OFF_WQ + 64] = wqT[0:128]
    wpack[:, OFF_WQ + 64:OFF_WQ + 128] = wqT[128:256]
    wkT = g["wm2"].reshape(64, 256).T
    wpack[:, OFF_WK:OFF_WK + 64] = wkT[0:128]
    wpack[:, OFF_WK + 64:OFF_WK + 128] = wkT[128:256]
    waT = g["wa"].reshape(256, 256).T
    wpack[:, OFF_WA:OFF_WA + 256] = waT[0:128]
    wpack[:, OFF_WA + 256:OFF_WA + 512] = waT[128:256]
    for name, off in (("bz_cw", OFF_BZW), ("th_cw", OFF_THW)):
        w3 = g[name].transpose(2, 3, 1, 0).reshape(9, 256, 64)
        for t in range(9):
            for c in range(2):
                wpack[:, off + (t * 2 + c) * 64:off + (t * 2 + c) * 64 + 64] = \
                    w3[t, c * 128:(c + 1) * 128]
    # conv_transpose flips the kernel: tap (di,dj) uses w[1-di, 1-dj]
    for name, off in (("bz_dw1", OFF_DW1BZ), ("th_dw1", OFF_DW1TH)):
        d1 = g[name].reshape(4, 64, 64)[::-1]
        for t in range(4):
            wpack[0:64, off + t * 64:off + (t + 1) * 64] = d1[t]
    for name, off in (("bz_dw2", OFF_DW2BZ), ("th_dw2", OFF_DW2TH)):
        d2 = g[name].transpose(2, 0, 1, 3).reshape(64, 4)[:, ::-1]
        wpack[0:64, off:off + 4] = d2
        wpack[64:128, off + 4:off + 8] = d2
    wpack[:, OFF_ONESR:OFF_ONESR + 128] = 1.0

    bpack = np.zeros((128, BCOLS), np.float32)
    bpack[0:64, BP_BQ] = g["bm1"]
    bpack[0:64, BP_BK] = g["bm2"]
    for pre, (cs1, cb1, cs2, cb2, cdb2) in (
            ("bz", (BP_BZ_S1, BP_BZ_B1, BP_BZ_S2, BP_BZ_B2, BP_BZ_DB2)),
            ("th", (BP_TH_S1, BP_TH_B1, BP_TH_S2, BP_TH_B2, BP_TH_DB2))):
        inv1 = g[f"{pre}_g1"] / np.sqrt(g[f"{pre}_v1"] + EPS)
        bpack[0:64, cs1] = inv1
        bpack[0:64, cb1] = g[f"{pre}_b1"] - g[f"{pre}_m1"] * inv1
        inv2 = g[f"{pre}_g2"] / np.sqrt(g[f"{pre}_v2"] + EPS)
        bpack[0:64, cs2] = inv2
        bpack[64:128, cs2] = inv2
        b2v = g[f"{pre}_b2"] + (g[f"{pre}_db1"] - g[f"{pre}_m2"]) * inv2
        bpack[0:64, cb2] = b2v
        bpack[64:128, cb2] = b2v
        bpack[0:8, cdb2] = float(g[f"{pre}_db2"][0])
    return g, wpack, bpack


def kernel(**inputs):
    global LAST_RESULTS
    if "nc" not in _CACHE:
        _CACHE["nc"] = _build()
    nc = _CACHE["nc"]
    g, wpack, bpack = _prep(inputs)
    x = g["x"]  # [4, 256, 64, 64]

    in_maps = []
    for core in range(8):
        b, s = core % 4, core // 4
        xq = np.zeros((256, 34, 64), np.float32)
        xpad = np.zeros((256, 34, 66), np.float32)
        qmask = np.ones((1, NQ), np.float32)
        if s == 0:
            xq[:, 1:34] = x[b][:, 0:33]
            xpad[:, 1:34, 1:65] = x[b][:, 0:33]
            qmask[0, 0:64] = 0.0
        else:
            xq[:, 0:33] = x[b][:, 31:64]
            xpad[:, 0:33, 1:65] = x[b][:, 31:64]
            qmask[0, 33 * 64:] = 0.0
        in_maps.append({"xin": np.ascontiguousarray(x[b]).astype(np.float16),
                        "xq": xq.astype(np.float16), "xpad": xpad.astype(np.float16),
                        "qmask": qmask, "wpack": wpack.astype(np.float16),
                        "bpack": bpack, "ba": g["ba"].astype(np.float16)})

    br = run_bass_kernel_spmd(
        nc, in_maps, core_ids=list(range(8)),
        trace=os.environ.get("KERNEL_TRACE", "0") == "1")
    LAST_RESULTS = br

    out = np.zeros((4, 3, 256, 256), np.float32)
    for core in range(8):
        b, s = core % 4, core // 4
        raw = br.results[core]["out"].reshape(3, 2, 2, 2, 4, 2, 8, 64)
        # [ch, i, ei, ej, blk, p, r', c]; tap t=(p,i) ->
        # rows (blk, r', p, ei), cols (c, i, ej)
        half = raw.transpose(0, 4, 6, 5, 2, 7, 1, 3).reshape(3, 128, 256)
        out[b, :, 128 * s:128 * (s + 1), :] = half
    return out


# revision 81
# speedup vs baseline: 1.0024x; 1.0024x over previous
"""DBHead (non-local attention + binarize/threshold conv branches) on 8 trn2 cores.

Sharding: 8 shards = 4 batch x 2 row-halves. Core (b, s) computes output rows
[128s, 128s+128) of the [3, 256, 256] map for batch b. All per-core variation
(which rows, halo padding, query-row masking) is pushed into host-prepared
input data so ONE SPMD program serves all 8 cores.

Final schedule (TimelineSim 179.9us/core vs 235.9us baseline, HW-verified
rel err 1.8e-3):
- everything DMA'd straight into f16 tiles (no staging/conversion pass);
  all weight/activation matmuls run f16 at full PE rate. Only exp outputs
  (range ~e^35) and V stay f32r.
- threshold-branch inputs load first; junk warm-up matmuls (high priority)
  ramp the PE clock while those DMAs fly.
- Act-engine ops are the scarce resource after the PE: every PSUM
  eviction is batched 2 banks per Act op (e1/e2 pairs, V quads, score
  pairs, 8-partition output strips) so the Act period stays under the
  PE period for each pipeline.
- attention runs as one flat software-pipelined stream over all (q-block,
  key-group) stages; S@V lags score/exp by 2 stages and a 64-wide pacing
  matmul per stage keeps the PE period just above the Act exp period so
  the exp pipeline never drifts into its PSUM WAR. The softmax
  denominator chain (DVE/Pool accumulate, gpsimd partition_all_reduce,
  reciprocal * query mask) runs entirely off the PE path; the final
  q-block's chain lives in long-lived tiles behind a fast unnormalized
  PSUM copy so no phase-3 pool barrier waits on it.
- ConvT taps packed two-per-matmul (M=64 -> 128) in dw1 and dw2; the
  binarize-threshold diff + sigmoid work is deferred off the Act critical
  path, the threshold map never leaves SBUF, and each output channel
  ships as ONE DMA descriptor (DRAM layout = SBUF accumulator layout).
"""
import sys, os
sys.path.insert(0, "/opt/trn_rl_repo")
import numpy as np
from contextlib import ExitStack

import concourse.bass as bass
import concourse.tile as tile
from concourse import mybir, bacc
from concourse.bass_utils import run_bass_kernel_spmd
from concourse import bass_isa

F32 = mybir.dt.float32
F32R = mybir.dt.float32r
F16 = mybir.dt.float16
AFT = mybir.ActivationFunctionType
ALU = mybir.AluOpType

EPS = 1e-5
NQ = 2176  # 34 rows x 64 cols of query positions (33 real + 1 zero halo row)
QBLOCKS = [(0, 512), (512, 512), (1024, 512), (1536, 512), (2048, 128)]
NKC = 32  # key chunks of 128 over 4096 positions

# wpack column offsets (f16, rows = contraction dim on partitions).
# Order = load priority: threshold branch first, then binarize, then attn.
OFF_THW = 0        # 18 x 64 (tap*2+chunk)
OFF_DW1TH = 1152   # 2 tap-pairs x 128 (rows 0:64)
OFF_DW2TH = 1408   # 8 cols block-diag (rows 0:128)
OFF_BZW = 1416
OFF_DW1BZ = 2568
OFF_DW2BZ = 2824
OFF_WQ = 2832      # 2 chunks x 64
OFF_WK = 2960
OFF_WA = 3088      # 2 chunks x 256
OFF_ONESR = 3600   # 128 cols of ones (row0 = K=1 bcast lhsT)
WCOLS = 3728

# bpack columns
BP_BQ, BP_BK = 0, 1
BP_BZ_S1, BP_BZ_B1, BP_BZ_S2, BP_BZ_B2 = 2, 3, 4, 5
BP_TH_S1, BP_TH_B1, BP_TH_S2, BP_TH_B2 = 6, 7, 8, 9
BP_BZ_DB2, BP_TH_DB2 = 10, 11
BP_BZ_BS1, BP_TH_BS1 = 12, 13
BCOLS = 16

_CACHE = {}
LAST_RESULTS = None


def _branch_ir(nc, tc, wr, bpk, hc, pcv, pct, ppt, pads, offw3, bs1, b1,
               offdw1, s2, b2, offdw2, db2, strip_fn, pre_blk_fn=None,
               blk_order=(0, 1, 2, 3), late_wait_ms=None):
    """Conv3x3+BN+ReLU, ConvT(2x2) x2 pipeline over 4 blocks of 8 rows.
    strip_fn(blk, pt, last) consumes each finished [8, 2, 512] PSUM strip
    pair (partitions 0:4 = dw1-tap 2p*4.., free = (p, col)).
    pre_blk_fn() can inject deferred work between blocks.
    """
    h1cs, h2cs = {}, {}

    def stage_a(blk, act_h1=False):  # conv3x3 into PSUM + BN/ReLU eviction
        cv = pcv.tile([64, 512], F32, tag="cv", name="cv")
        for t in range(9):
            ky, kx = t // 3, t % 3
            for c in range(2):
                o = offw3 + (t * 2 + c) * 64
                nc.tensor.matmul(
                    cv[:], lhsT=wr[:, o:o + 64],
                    rhs=pads[c][:, blk * 8 + ky:blk * 8 + ky + 8, kx:kx + 64],
                    start=(t == 0 and c == 0), stop=(t == 8 and c == 1))
        h1c = hc.tile([64, 512], F16, tag="h1c", name="h1c")
        # BN1 is folded into the dw1 weights (inv-std > 0), so the
        # eviction is one fused add+relu op: max(cv + b1/s1, 0)
        if act_h1:
            nc.scalar.activation(h1c[:], cv[:], AFT.Relu,
                                 bias=bpk[0:64, bs1:bs1 + 1])
        else:
            nc.vector.tensor_scalar(h1c[:], cv[:], bpk[0:64, bs1:bs1 + 1],
                                    0.0, ALU.add, ALU.max)
        h1cs[blk] = h1c

    def stage_b(blk):  # dw1 tap-pair matmuls + one paired BN/ReLU eviction
        ct = pct.tile([128, 2, 512], F32, tag="ct", name="ct")
        for p in range(2):
            o = offdw1 + p * 128
            nc.tensor.matmul(ct[:, p, :], lhsT=wr[0:64, o:o + 128],
                             rhs=h1cs[blk][:], start=True, stop=True)
        h2c = hc.tile([128, 2, 512], F16, tag="h2c", name="h2c")
        nc.scalar.activation(h2c[:], ct[:], AFT.Relu,
                             bias=bpk[:, b2:b2 + 1],
                             scale=bpk[:, s2:s2 + 1])
        h2cs[blk] = h2c

    def stage_c(blk, last=False):  # dw2 matmuls + output strip
        if pre_blk_fn is not None:
            pre_blk_fn()
        pt = ppt.tile([8, 2, 512], F32, tag="pt", name="pt")
        for p in range(2):
            nc.tensor.matmul(pt[:, p, :], lhsT=wr[:, offdw2:offdw2 + 8],
                             rhs=h2cs[blk][:, p, :], start=True, stop=True)
        strip_fn(blk, pt, last)

    # software-pipelined emission: the PE always has a later block's conv
    # between a dw1 and the dw2 that consumes its Act-evicted h2c.
    o = blk_order
    act_h1 = blk_order[0] != 0  # reversed = binarize branch after attention
    stage_a(o[0], act_h1); stage_a(o[1], act_h1); stage_b(o[0])
    stage_a(o[2]); stage_c(o[0]); stage_b(o[1])
    if late_wait_ms is not None:
        # keep the scheduler from hoisting this conv (whose input is the
        # final attention rows) ahead of the rest of the branch
        with tc.tile_wait_until(late_wait_ms):
            stage_a(o[3])
    else:
        stage_a(o[3])
    stage_c(o[1]); stage_b(o[2]); stage_b(o[3])
    stage_c(o[2]); stage_c(o[3], last=True)


def _build():
    nc = bacc.Bacc("TRN2", target_bir_lowering=False, debug=False, num_devices=8)
    xin_d = nc.dram_tensor("xin", [256, 64, 64], F16, kind="ExternalInput").ap()
    xq_d = nc.dram_tensor("xq", [256, 34, 64], F16, kind="ExternalInput").ap()
    xpad_d = nc.dram_tensor("xpad", [256, 34, 66], F16, kind="ExternalInput").ap()
    qm_d = nc.dram_tensor("qmask", [1, NQ], F32, kind="ExternalInput").ap()
    wp_d = nc.dram_tensor("wpack", [128, WCOLS], F16, kind="ExternalInput").ap()
    bp_d = nc.dram_tensor("bpack", [128, BCOLS], F32, kind="ExternalInput").ap()
    ba_d = nc.dram_tensor("ba", [256], F16, kind="ExternalInput").ap()
    out_d = nc.dram_tensor("out", [3, 8, 4, 2, 512], F32, kind="ExternalOutput").ap()

    with tile.TileContext(nc) as tc, ExitStack() as ctx:
        cp = ctx.enter_context(tc.tile_pool(name="const", bufs=1))
        pp = ctx.enter_context(tc.tile_pool(name="pads", bufs=1))

        wr = cp.tile([128, WCOLS], F16)
        bpk = cp.tile([128, BCOLS], F32)
        bar = cp.tile([1, 256], F16)
        qm128 = cp.tile([128, NQ], F32)
        zc = cp.tile([128, 34], F32)
        # threshold map kept on-chip: partitions = 4*(tap-in-pair i) + convT2
        # tap, free = (blk, dw1-pair p, col); output tap t = 2p + i
        Ts = cp.tile([8, 4, 2, 512], F32)

        xpr = [pp.tile([128, 34, 66], F16, tag=f"xp{c}", name=f"xpr{c}")
               for c in range(2)]
        xnp = [pp.tile([128, 34, 66], F16, tag=f"xn{c}", name=f"xnp{c}")
               for c in range(2)]
        # the FINAL attention q-block's normalization chain lives here (not
        # in a phase-2-scoped pool) so the phase-3 pool-close barriers never
        # wait on it; its PSUM is freed by one fast copy into xnr2.
        xnr2 = pp.tile([128, 2, 512], F32, tag="xnr2", name="xnr2")
        A2f = pp.tile([128, 2, 512], F32, tag="A2f", name="A2f")
        Fs2 = pp.tile([128, 512], F32, tag="Fs2", name="Fs2")
        Rs2 = pp.tile([128, 512], F32, tag="Rs2", name="Rs2")
        Ci2 = pp.tile([128, 512], F32, tag="Ci2", name="Ci2")
        Cm2 = pp.tile([128, 512], F32, tag="Cm2", name="Cm2")

        # ---- loads, in priority order (threshold-branch inputs first).
        # All issued from the idle SP engine. ----
        qm = cp.tile([1, NQ], F32)
        nc.sync.dma_start(wr[:, 0:OFF_BZW], wp_d[:, 0:OFF_BZW])
        for c in range(2):
            sl = slice(c * 128, (c + 1) * 128)
            nc.sync.dma_start(xpr[c][:], xpad_d[sl])
        nc.sync.dma_start(bpk[:], bp_d[:])
        nc.sync.dma_start(wr[:, OFF_BZW:], wp_d[:, OFF_BZW:])
        nc.sync.dma_start(qm[:], qm_d[:])
        nc.sync.dma_start(bar[:], bass.AP(tensor=ba_d.tensor, offset=ba_d.offset,
                                          ap=[[0, 1]] + [list(a) for a in ba_d.ap]))

        nc.vector.memset(zc[:], 0.0)
        # ba broadcast to all partitions x 4 value-chunks, for the DVE-side
        # V evictions that fold the bias in (Act bias is per-partition only)
        baf4 = cp.tile([1, 4, 256], F32)
        bav4 = cp.tile([128, 4, 256], F32)
        for u in range(4):
            nc.vector.tensor_copy(baf4[:, u, :], bar[:])
        nc.gpsimd.partition_broadcast(bav4[:], baf4[:])

        # ---- PE clock warm-up: junk matmuls while the first DMAs fly ----
        with tc.high_priority(), \
             tc.tile_pool(name="wrm", bufs=1) as wp_, \
             tc.tile_pool(name="wps", bufs=2, space="PSUM") as wps:
            jz = wp_.tile([128, 512], F16)
            nc.vector.memset(jz[:], 0.0)
            jo = wp_.tile([1, 16], F32)
            for i in range(9):
                wt = wps.tile([128, 512], F32, tag="w", name="wt")
                nc.tensor.matmul(wt[:], lhsT=jz[:, 0:128], rhs=jz[:],
                                 start=True, stop=True)
                # keep the BIR verifier happy: every PSUM write needs a reader
                nc.vector.tensor_copy(jo[:, i:i + 1], wt[0:1, 0:1])

        with tc.tile_pool(name="att", bufs=1) as ap_:
            e1r = ap_.tile([64, NQ], F16)
            e2r = ap_.tile([64, 4096], F16)
            V = ap_.tile([128, NKC, 256], F32R)

            with tc.tile_pool(name="xr", bufs=1) as xp:
                xr = [xp.tile([128, 64, 64], F16, tag=f"xr{c}", name=f"xr{c}")
                      for c in range(2)]
                xqr = [xp.tile([128, 34, 64], F16, tag=f"xq{c}", name=f"xqr{c}")
                       for c in range(2)]
                for c in range(2):
                    sl = slice(c * 128, (c + 1) * 128)
                    nc.sync.dma_start(xqr[c][:], xq_d[sl])
                for c in range(2):
                    sl = slice(c * 128, (c + 1) * 128)
                    nc.sync.dma_start(xr[c][:], xin_d[sl])

                # threshold branch: independent of attention; fills the PE
                # while attention inputs stream in. Strips go to DRAM (f32)
                # and to the on-chip f16 Tt for the binarize diff later.
                with tc.tile_pool(name="hct", bufs=2) as hct, \
                     tc.tile_pool(name="pcv0", bufs=2, space="PSUM") as pcv0, \
                     tc.tile_pool(name="pct0", bufs=2, space="PSUM") as pct0, \
                     tc.tile_pool(name="ppt0", bufs=1, space="PSUM") as ppt0:
                    def th_strip(blk, pt, last):
                        nc.scalar.activation(
                            Ts[:, blk, :, :], pt[:], AFT.Sigmoid,
                            bias=bpk[0:8, BP_TH_DB2:BP_TH_DB2 + 1])
                    _branch_ir(nc, tc, wr, bpk, hct, pcv0, pct0, ppt0, xpr,
                               OFF_THW, BP_TH_BS1, BP_TH_B1, OFF_DW1TH,
                               BP_TH_S2, BP_TH_B2, OFF_DW2TH, BP_TH_DB2,
                               th_strip)
                    # DRAM layout matches the SBUF tile: one descriptor
                    nc.scalar.dma_start(out_d[1], Ts[:])
                # zero borders of xn pads (cols 0/65); rows are written later
                for c in range(2):
                    for col in (0, 65):
                        nc.vector.tensor_copy(
                            xnp[c][:, :, col:col + 1],
                            zc[:].rearrange("p (r o) -> p r o", o=1))

                xr_f = [t[:].rearrange("p r c2 -> p (r c2)") for t in xr]
                xq_f = [t[:].rearrange("p r c2 -> p (r c2)") for t in xqr]

                # ---- phase 1: e1 (queries), e2 (keys), V (values).
                # Two 512-chunks per PSUM tile so each Act eviction covers
                # 2 banks and the Act period stays under the PE period. ----
                with tc.tile_pool(name="pe", bufs=2, space="PSUM") as pe, \
                     tc.tile_pool(name="pv", bufs=2, space="PSUM") as pv:
                    def proj(dst, src_f, off, bias_col, k0, w2):
                        # one [64, 2, 512] psum tile covering cols k0:k0+w2
                        p = pe.tile([64, 2, 512], F32, tag="pe", name="pe_t")
                        for h in range(2):
                            hw = min(512, w2 - h * 512)
                            if hw <= 0:
                                break
                            for c in range(2):
                                o = off + c * 64
                                nc.tensor.matmul(
                                    p[:, h, :hw], lhsT=wr[:, o:o + 64],
                                    rhs=src_f[c][:, k0 + h * 512:k0 + h * 512 + hw],
                                    start=(c == 0), stop=(c == 1))
                        nc.scalar.activation(
                            dst[:, k0:k0 + w2].rearrange("p (h c2) -> p h c2",
                                                         c2=512)
                            if w2 == 1024 else dst[:, k0:k0 + w2],
                            p[:, :, :] if w2 == 1024 else p[:, 0, :w2],
                            AFT.Prelu, bias=bpk[0:64, bias_col:bias_col + 1],
                            alpha=0.25)
                    for k0 in range(0, 2048, 1024):
                        proj(e1r, xq_f, OFF_WQ, BP_BQ, k0, 1024)
                    proj(e1r, xq_f, OFF_WQ, BP_BQ, 2048, 128)
                    for q in range(NKC // 4):  # V in quads: 4 chunks/eviction
                        act_q = q % 2 == 0
                        p = pv.tile([128, 4, 256], F32, tag="pv", name="pv_t")
                        for u in range(4):
                            j = 4 * q + u
                            for c in range(2):
                                o = OFF_WA + c * 256
                                nc.tensor.matmul(p[:, u, :],
                                                 lhsT=xr_f[c][:, j * 128:(j + 1) * 128],
                                                 rhs=wr[:, o:o + 256],
                                                 start=(c == 0),
                                                 stop=(c == 1 and not act_q))
                            if act_q:
                                # Act's Prelu bias is per-partition only, so
                                # these quads take the bias via a ones-matmul
                                nc.tensor.matmul(
                                    p[:, u, :],
                                    lhsT=wr[0:1, OFF_ONESR:OFF_ONESR + 128],
                                    rhs=bar[:], start=False, stop=True)
                        if act_q:
                            nc.scalar.activation(V[:, 4 * q:4 * q + 4, :], p[:],
                                                 AFT.Prelu, alpha=0.25)
                        else:
                            # DVE eviction folds the bias in: z = p + ba
                            # (broadcast), then PReLU(z) == max(0.25z, z).
                            # One PSUM input per instruction (HW rule).
                            sl = V[:, 4 * q:4 * q + 4, :]
                            nc.vector.scalar_tensor_tensor(
                                sl, p[:], 1.0, bav4[:], ALU.mult, ALU.add)
                            nc.vector.scalar_tensor_tensor(
                                sl, sl, 0.25, sl, ALU.mult, ALU.max)
                    # e2 last: its first chunk-pair unblocks the score
                    # matmuls while the remaining evictions drain
                    for k0 in range(0, 4096, 1024):
                        proj(e2r, xr_f, OFF_WK, BP_BK, k0, 1024)

            # ---- phase 2: attention, one flat pipelined stream over all
            # (q-block, key-group) stages; consume side lags produce side
            # by 2 stages so the PE never waits on exp. Big blocks use
            # 2-key-chunk stages; the small tail block packs 8 key chunks
            # per stage so its exp overhead stays amortized. ----
            nc.gpsimd.partition_broadcast(qm128[:], qm[:])
            stages = []
            for q0, w in QBLOCKS:
                kper = 2 if w > 128 else 8
                for g in range(NKC // kper):
                    stages.append((q0, w, list(range(kper * g, kper * g + kper))))
            LAG = 2
            with tc.tile_pool(name="psc", bufs=2, space="PSUM") as psc, \
                 tc.tile_pool(name="pxn", bufs=2, space="PSUM") as pxn, \
                 tc.tile_pool(name="eb", bufs=10) as eb, \
                 tc.tile_pool(name="rc", bufs=2) as rc:
                Es = {}
                cur = {}
                for i in range(len(stages) + LAG):
                    if i < len(stages):
                        q0, w, ks = stages[i]
                        sc = psc.tile([128, 2, 512], F32, tag="sc", name="sc")
                        if w == 512:
                            # pacing matmul (+27ns, overwritten by the real
                            # score below): keeps the PE stage period just
                            # above the Act exp period so the exp pipeline
                            # never drifts into the PSUM WAR stall
                            nc.tensor.matmul(sc[:, 0, 0:64],
                                             lhsT=e2r[:, 0:128],
                                             rhs=e1r[:, q0:q0 + 64],
                                             start=True, stop=True)
                        for n, j in enumerate(ks):
                            u, m = n % 2, n // 2
                            nc.tensor.matmul(sc[:, u, m * w:(m + 1) * w],
                                             lhsT=e2r[:, j * 128:(j + 1) * 128],
                                             rhs=e1r[:, q0:q0 + w],
                                             start=True, stop=True)
                        E = eb.tile([128, 2, 512], F32R, tag="E", name="E")
                        wv = w * len(ks) // 2
                        nc.scalar.activation(E[:, :, :wv], sc[:, :, :wv], AFT.Exp)
                        Es[i] = E
                    if i >= LAG:
                        k = i - LAG
                        q0, w, ks = stages[k]
                        wv = w * len(ks) // 2
                        first = ks[0] == 0
                        if first:
                            cur["xn"] = pxn.tile([128, 2, 512], F32, tag="xn",
                                                 name="xnps")
                            cur["A"] = (A2f if q0 == QBLOCKS[-1][0] else
                                        rc.tile([128, 2, 512], F32, tag="A",
                                                name="A"))
                        xn_ps, A = cur["xn"], cur["A"]
                        E = Es.pop(k)
                        final = (ks[-1] == NKC - 1 and q0 == QBLOCKS[-1][0])
                        for n, j in enumerate(ks):
                            u, m = n % 2, n // 2
                            for t in range(2):
                                nc.tensor.matmul(
                                    xn_ps[:, t, :w],
                                    lhsT=V[:, j, t * 128:(t + 1) * 128],
                                    rhs=E[:, u, m * w:m * w + w],
                                    start=(j == 0), stop=(j == NKC - 1))
                        if final:
                            # evict unnormalized BEFORE the remaining adds so
                            # the phase-3 PSUM pool barrier releases at once
                            with tc.high_priority():
                                nc.vector.tensor_copy(xnr2[:, :, :w],
                                                      xn_ps[:, :, :w])
                        # DVE adds are ~3.5x faster than Pool's: ~1 in 5
                        # accumulates goes to Pool, evenly spread
                        gi = ks[0] // len(ks)
                        eng = nc.gpsimd if gi % 5 == 2 else nc.vector
                        if first:
                            eng.tensor_copy(A[:, :, :wv], E[:, :, :wv])
                        else:
                            eng.tensor_add(A[:, :, :wv], A[:, :, :wv],
                                           E[:, :, :wv])
                        if ks[-1] == NKC - 1 and not final:
                            rows, r0 = w // 64, q0 // 64
                            Fs = rc.tile([128, 512], F32, tag="Fs", name="Fs")
                            nc.vector.tensor_add(Fs[:, :wv], A[:, 0, :wv],
                                                 A[:, 1, :wv])
                            # fold sub-slots (small tail block only)
                            while wv > w:
                                wv //= 2
                                nc.vector.tensor_add(Fs[:, :wv], Fs[:, :wv],
                                                     Fs[:, wv:2 * wv])
                            Rs = rc.tile([128, 512], F32, tag="Rs", name="Rs")
                            nc.gpsimd.partition_all_reduce(
                                Rs[:, :w], Fs[:, :w], 128, bass_isa.ReduceOp.add)
                            Ci = rc.tile([128, 512], F32, tag="Ci", name="Ci")
                            nc.vector.reciprocal(Ci[:, :w], Rs[:, :w])
                            Cm = rc.tile([128, 512], F32, tag="Cm", name="Cm")
                            nc.vector.tensor_mul(Cm[:, :w], Ci[:, :w],
                                                 qm128[:, q0:q0 + w])
                            for t in range(2):
                                # gpsimd cannot touch PSUM: both on DVE
                                nc.vector.tensor_mul(
                                    xnp[t][:, r0:r0 + rows, 1:65],
                                    xn_ps[:, t, :w].rearrange(
                                        "p (r c2) -> p r c2", c2=64),
                                    Cm[:, :w].rearrange(
                                        "p (r c2) -> p r c2", c2=64))

            # final q-block's normalization: emitted after the phase-2 pools
            # close (only long-lived tiles involved) so no pool-close barrier
            # waits on it; the binarize branch reads these rows ~10us later.
            q0f, wf = QBLOCKS[-1]
            rowsf, r0f = wf // 64, q0f // 64
            wvf = wf * (8 if wf == 128 else 2) // 2
            nc.vector.tensor_add(Fs2[:, :wvf], A2f[:, 0, :wvf],
                                 A2f[:, 1, :wvf])
            while wvf > wf:  # fold sub-slots (small tail block only)
                wvf //= 2
                nc.vector.tensor_add(Fs2[:, :wvf], Fs2[:, :wvf],
                                     Fs2[:, wvf:2 * wvf])
            nc.gpsimd.partition_all_reduce(Rs2[:, :wf], Fs2[:, :wf], 128,
                                           bass_isa.ReduceOp.add)
            nc.vector.reciprocal(Ci2[:, :wf], Rs2[:, :wf])
            nc.vector.tensor_mul(Cm2[:, :wf], Ci2[:, :wf],
                                 qm128[:, q0f:q0f + wf])
            for t in range(2):
                nc.vector.tensor_mul(
                    xnp[t][:, r0f:r0f + rowsf, 1:65],
                    xnr2[:, t, :wf].rearrange("p (r c2) -> p r c2", c2=64),
                    Cm2[:, :wf].rearrange("p (r c2) -> p r c2", c2=64))

        # ---- phase 3: binarize branch (att pool closed; SBUF freed).
        # P strips stream out as they finish; the threshold diff + final
        # sigmoid (B channel) is deferred off the Act critical path. ----
        with tc.tile_pool(name="hc", bufs=4) as hc, \
             tc.tile_pool(name="pb", bufs=1) as pb, \
             tc.tile_pool(name="pcv", bufs=2, space="PSUM") as pcv, \
             tc.tile_pool(name="pct", bufs=1, space="PSUM") as pct, \
             tc.tile_pool(name="ppt", bufs=2, space="PSUM") as ppt:
            Pa = pb.tile([8, 4, 2, 512], F32, tag="Pa", name="Pa")
            Ba = pb.tile([8, 4, 2, 512], F32, tag="Ba", name="Ba")
            pending = []

            def flush_one():
                blk, dc = pending.pop(0)
                nc.scalar.activation(Ba[:, blk, :, :], dc[:], AFT.Sigmoid,
                                     scale=50.0)

            def bz_strip(blk, pt, last):
                if not last:
                    nc.scalar.activation(Pa[:, blk, :, :], pt[:], AFT.Sigmoid,
                                         bias=bpk[0:8, BP_BZ_DB2:BP_BZ_DB2 + 1])
                    dc = hc.tile([8, 2, 512], F32, tag="dc", name="dc")
                    nc.vector.tensor_sub(dc[:], Pa[:, blk, :, :],
                                         Ts[:, blk, :, :])
                    pending.append((blk, dc))
                    return
                # tail block: drain deferred sigmoids first (the row DMAs
                # below read every block), then per-pair pieces so the
                # final Act chain is short
                while pending:
                    flush_one()
                for p in range(2):
                    nc.scalar.activation(Pa[:, blk, p, :], pt[:, p, :],
                                         AFT.Sigmoid,
                                         bias=bpk[0:8, BP_BZ_DB2:BP_BZ_DB2 + 1])
                    dcp = hc.tile([8, 512], F32, tag="dcp", name="dcp")
                    nc.vector.tensor_sub(dcp[:], Pa[:, blk, p, :],
                                         Ts[:, blk, p, :])
                    nc.scalar.activation(Ba[:, blk, p, :], dcp[:], AFT.Sigmoid,
                                         scale=50.0)
                    if p == 1:
                        # all blocks complete: single-descriptor channels
                        nc.sync.dma_start(out_d[0], Pa[:])
                        nc.scalar.dma_start(out_d[2], Ba[:])

            def pre_blk():
                while len(pending) >= 1:
                    flush_one()

            _branch_ir(nc, tc, wr, bpk, hc, pcv, pct, ppt, xnp, OFF_BZW,
                       BP_BZ_BS1, BP_BZ_B1, OFF_DW1BZ, BP_BZ_S2, BP_BZ_B2,
                       OFF_DW2BZ, BP_BZ_DB2, bz_strip, pre_blk_fn=pre_blk,
                       blk_order=(0, 1, 2, 3))
            while pending:
                flush_one()

    nc.compile()
    return nc


def _prep(inputs):
    """Host-side parameter prep shared by all cores (numpy, tiny)."""
    g = {k: np.asarray(v, np.float32) for k, v in inputs.items()}
    wpack = np.zeros((128, WCOLS), np.float32)
    wqT = g["wm1"].reshape(64, 256).T
    wpack[:, OFF_WQ:OFF_WQ + 64] = wqT[0:128]
    wpack[:, OFF_WQ + 64:OFF_WQ + 128] = wqT[128:256]
    wkT = g["wm2"].reshape(64, 256).T
    wpack[:, OFF_WK:OFF_WK + 64] = wkT[0:128]
    wpack[:, OFF_WK + 64:OFF_WK + 128] = wkT[128:256]
    waT = g["wa"].reshape(256, 256).T
    wpack[:, OFF_WA:OFF_WA + 256] = waT[0:128]
    wpack[:, OFF_WA + 256:OFF_WA + 512] = waT[128:256]
    for name, off in (("bz_cw", OFF_BZW), ("th_cw", OFF_THW)):
        w3 = g[name].transpose(2, 3, 1, 0).reshape(9, 256, 64)
        for t in range(9):
            for c in range(2):
                wpack[:, off + (t * 2 + c) * 64:off + (t * 2 + c) * 64 + 64] = \
                    w3[t, c * 128:(c + 1) * 128]
    # conv_transpose flips the kernel: tap (di,dj) uses w[1-di, 1-dj]
    for name, off, pre in (("bz_dw1", OFF_DW1BZ, "bz"), ("th_dw1", OFF_DW1TH, "th")):
        inv1 = g[f"{pre}_g1"] / np.sqrt(g[f"{pre}_v1"] + EPS)
        d1 = g[name].reshape(4, 64, 64)[::-1] * inv1[None, :, None]
        for t in range(4):
            wpack[0:64, off + t * 64:off + (t + 1) * 64] = d1[t]
    for name, off in (("bz_dw2", OFF_DW2BZ), ("th_dw2", OFF_DW2TH)):
        d2 = g[name].transpose(2, 0, 1, 3).reshape(64, 4)[:, ::-1]
        wpack[0:64, off:off + 4] = d2
        wpack[64:128, off + 4:off + 8] = d2
    wpack[:, OFF_ONESR:OFF_ONESR + 128] = 1.0

    bpack = np.zeros((128, BCOLS), np.float32)
    bpack[0:64, BP_BQ] = g["bm1"]
    bpack[0:64, BP_BK] = g["bm2"]
    for pre, (cs1, cb1, cs2, cb2, cdb2) in (
            ("bz", (BP_BZ_S1, BP_BZ_B1, BP_BZ_S2, BP_BZ_B2, BP_BZ_DB2)),
            ("th", (BP_TH_S1, BP_TH_B1, BP_TH_S2, BP_TH_B2, BP_TH_DB2))):
        inv1 = g[f"{pre}_g1"] / np.sqrt(g[f"{pre}_v1"] + EPS)
        bpack[0:64, cs1] = inv1
        bpack[0:64, cb1] = g[f"{pre}_b1"] - g[f"{pre}_m1"] * inv1
        # BN1 folded: ReLU(s*cv+b) = s*ReLU(cv + b/s), s absorbed into dw1
        bpack[0:64, {"bz": BP_BZ_BS1, "th": BP_TH_BS1}[pre]] = \
            (g[f"{pre}_b1"] - g[f"{pre}_m1"] * inv1) / inv1
        inv2 = g[f"{pre}_g2"] / np.sqrt(g[f"{pre}_v2"] + EPS)
        bpack[0:64, cs2] = inv2
        bpack[64:128, cs2] = inv2
        b2v = g[f"{pre}_b2"] + (g[f"{pre}_db1"] - g[f"{pre}_m2"]) * inv2
        bpack[0:64, cb2] = b2v
        bpack[64:128, cb2] = b2v
        bpack[0:8, cdb2] = float(g[f"{pre}_db2"][0])
    return g, wpack, bpack


def kernel(**inputs):
    global LAST_RESULTS
    if "nc" not in _CACHE:
        _CACHE["nc"] = _build()
    nc = _CACHE["nc"]
    g, wpack, bpack = _prep(inputs)
    x = g["x"]  # [4, 256, 64, 64]

    in_maps = []
    for core in range(8):
        b, s = core % 4, core // 4
        xq = np.zeros((256, 34, 64), np.float32)
        xpad = np.zeros((256, 34, 66), np.float32)
        qmask = np.ones((1, NQ), np.float32)
        if s == 0:
            xq[:, 1:34] = x[b][:, 0:33]
            xpad[:, 1:34, 1:65] = x[b][:, 0:33]
            qmask[0, 0:64] = 0.0
        else:
            xq[:, 0:33] = x[b][:, 31:64]
            xpad[:, 0:33, 1:65] = x[b][:, 31:64]
            qmask[0, 33 * 64:] = 0.0
        in_maps.append({"xin": np.ascontiguousarray(x[b]).astype(np.float16),
                        "xq": xq.astype(np.float16), "xpad": xpad.astype(np.float16),
                        "qmask": qmask, "wpack": wpack.astype(np.float16),
                        "bpack": bpack, "ba": g["ba"].astype(np.float16)})

    br = run_bass_kernel_spmd(
        nc, in_maps, core_ids=list(range(8)),
        trace=os.environ.get("KERNEL_TRACE", "0") == "1")
    LAST_RESULTS = br

    out = np.zeros((4, 3, 256, 256), np.float32)
    for core in range(8):
        b, s = core % 4, core // 4
        raw = br.results[core]["out"].reshape(3, 2, 2, 2, 4, 2, 8, 64)
        # [ch, i, ei, ej, blk, p, r', c]; tap t=(p,i) ->
        # rows (blk, r', p, ei), cols (c, i, ej)
        half = raw.transpose(0, 4, 6, 5, 2, 7, 1, 3).reshape(3, 128, 256)
        out[b, :, 128 * s:128 * (s + 1), :] = half
    return out
